# revision 26
# baseline (speedup 1.0000x reference)
"""Fixed-point (MPC) 3x3 VALID conv2d, NHWC, f32 — Trainium2 Bass kernel. v8.8.

Semantics (bit-exact vs the jax reference, fixed_point=8, S=256):
    qx = round_half_even(x*S)/S ; qw = round_half_even(w*S)/S
    y  = conv2d_valid(qx, qw)   ; out = floor(y*S)/S

Strategy per core (data-parallel over batch, 4 images/core):
  - HOST pre-quantizes x to fp16 integers (round(x*256), |.|<2048 so
    exact in fp16) and builds an image-PACKED blocked-transposed
    layout: stripe b holds w-columns [6b, 6b+8) as partitions
    (8dw,16c), with the 4 images contiguous along the free axis
    (4*224h + 2 pad cols), so the 888 output rows per stripe are
    covered by SEVEN M=128 stationary windows, all with
    NumWeights==128 => Fast Weight Load enabled on every matmul.
    Seam partitions (h' in {222,223}) are garbage, sliced off by host.
  - DEDUPED input: stripes 1..36 are uploaded WITHOUT their first 2
    w-columns (the 32-partition overlap with the previous stripe);
    those are reconstructed on-chip by DVE partition-crossing copies
    (no DMA-fabric cost).  HBM reads: 6.43MB/core instead of 8.56MB.
  - FLIPPED banded matmul: lhsT = xq window [128, 128] (stationary,
    FWL), rhs = wb[kh] [128, 96] fp16 (moving, PRE-SCALED by 1/256 so
    PSUM holds y on the 2^-8 grid exactly); 3 kh taps accumulate in
    PSUM -> psy[row part, (6w',16k)] in store orientation.
  - floor -> int16 in ONE op: int16_cast_RNE(psy - 255/512)
    == floor(psy*256) exactly (no ties; HW cast verified RNE).
    Split DVE (windows 0,2,4,6) / ACT (1,3,5) to stay under PE pace.
    int16 store halves output traffic (6.37MB/core incl. seam pad).
  - stores: output DRAM padded to 224 rows/img so each window stores
    as ONE contiguous [128, cols] DMA; 3 column waves on the gpsimd
    queue overlap compute (DMA triggers cost ~0.6us engine time each,
    so big and few).
  - HOST converts y_int16 -> f32 * (1/256) (exact) and gathers.

Engine budget per core-rep: PE 777 FWL matmuls ~37us (pacer);
DMA ~12.8MB across two lanes ~27+25us overlapped; DVE floors+copies
~25us; ACT floors ~15us.  Measured steady-state ~43.6us/rep.
"""

import numpy as np

import concourse.mybir as mybir
from concourse import bass, tile

N_CORES = 8
B_FULL = 32
B_CORE = B_FULL // N_CORES  # 4 images per core
H = W = 224
C = K = 16
HO = WO = 222

F32 = mybir.dt.float32
F16 = mybir.dt.float16
I16 = mybir.dt.int16

FLOOR_C = -255.0 / 512.0  # RNE(v + FLOOR_C) == floor(v) for v on 2^-8 grid

N_BLK = 37       # 37 blocks x 6 output w's = 222
GRP = 5          # blocks per PSUM group: 37 = 7*5 + 2
PACK = B_CORE * H          # 896 packed rows per block stripe
XSTRIDE = PACK + 2         # 898: 2 zero pad cols (window 6 taps 1,2)
XCOL = N_BLK * XSTRIDE     # 33226 columns total
N_WIN = 7                  # ceil(896 / 128) stationary windows

# block groups; input DMA split == groups.  First splits are small so
# the PE can start ~3us earlier; PSUM tiles are sized GRP=5 blocks.
groups = [(0, 2), (2, 3), (5, 5), (10, 5), (15, 5), (20, 5), (25, 5),
          (30, 5), (35, 2)]

# window w -> contiguous valid-output runs (i0, n, img, h0):
# partition i of window w is global packed row g = 128*w + i,
# img = g // 224, h' = g % 224, valid while h' <= 221.
WMAP = []
for _w in range(N_WIN):
    runs = []
    _i = 0
    while _i < 128:
        _g = 128 * _w + _i
        _img, _hp = divmod(_g, 224)
        if _hp >= HO:
            _i += 1
            continue
        _n = min(128 - _i, HO - _hp)
        runs.append((_i, _n, _img, _hp))
        _i += _n
    WMAP.append(runs)


def _split_multi_waits(nc):
    """The installed walrus only encodes ONE sync wait per instruction.
    Hoist extra waits onto NoOps inserted just before, same engine."""
    for f in nc.m.functions:
        for bb in f.blocks:
            new_list = []
            changed = False
            for ins in bb.instructions:
                si = ins.sync_info
                if si is not None and si.on_wait and len(si.on_wait) > 1:
                    waits = list(si.on_wait)
                    for wt in waits[:-1]:
                        nop = mybir.InstNoOp(
                            name=f"NOPW-{nc.next_id()}", ins=[], outs=[]
                        )
                        nop.engine = ins.engine
                        nop.sync_info = mybir.SyncInfo(on_wait=[wt], on_update=[])
                        new_list.append(nop)
                    ins.sync_info = mybir.SyncInfo(
                        on_wait=[waits[-1]], on_update=list(si.on_update or [])
                    )
                    changed = True
                new_list.append(ins)
            if changed:
                bb.instructions = new_list


def _build_nc(stage_limit: int = 7, reps: int = 1):
    # stage_limit: 1=loads 4=+conv 6=+floor 7=+store (full kernel).
    # reps>1 repeats the whole pipeline in-NEFF (timing harness only).
    nc = bass.Bass("TRN2", num_devices=N_CORES)
    # deduped input: stripe 0 full [128p, 898]; stripes 1..36 only their
    # NEW 6 w-columns (partitions 32..127).  The 2 overlap w-columns
    # (partitions 0..31) are reconstructed on-chip from the previous
    # stripe's partitions 96..127 via SBUF->SBUF DMA (no HBM traffic):
    # 8.56MB -> 6.43MB HBM reads per core.
    xq0_d = nc.dram_tensor("xq0", [128, XSTRIDE], F16, kind="ExternalInput")
    xqd_d = nc.dram_tensor(
        "xqd", [96, (N_BLK - 1) * XSTRIDE], F16, kind="ExternalInput"
    )
    wb_d = nc.dram_tensor("wb", [3, 128, 96], F16, kind="ExternalInput")
    # output padded to 224 rows/img: window w's 128 partitions land as
    # contiguous packed rows 128w..128w+127 (seam rows 222/223 hold
    # garbage; the host slices them off) -> ONE dma_start per store
    y_d = nc.dram_tensor("y", [B_CORE * H, WO * K], I16,
                         kind="ExternalOutput")

    add = mybir.AluOpType.add

    with tile.TileContext(nc) as tc:
        with (
            tc.tile_pool(name="consts", bufs=1) as consts,
            tc.tile_pool(name="xq", bufs=1) as xq_pool,
            tc.tile_pool(name="st", bufs=2) as st_pool,
            tc.tile_pool(name="psy", bufs=8, space="PSUM") as ps_pool,
        ):
            wtiles = []
            for kh in range(3):
                wt = consts.tile([128, 96], F16, tag=f"w{kh}")
                nc.sync.dma_start(out=wt[:], in_=wb_d[kh])
                wtiles.append(wt)

            for rp in range(reps):
                # ---- input DMA: one stripe-aligned split per group,
                # deduped loads + overlap reconstruction copies ----
                xts = []  # per group: (tile, block base)
                for gi, (b0, nb) in enumerate(groups):
                    t = xq_pool.tile([128, nb * XSTRIDE], F16, tag=f"xq{gi}")
                    if gi == 0:
                        # stripe 0 arrives full; stripes 1.. deduped
                        nc.sync.dma_start(out=t[:, :XSTRIDE], in_=xq0_d[:])
                        nc.sync.dma_start(
                            out=t[32:, XSTRIDE : nb * XSTRIDE],
                            in_=xqd_d[:, : (nb - 1) * XSTRIDE],
                        )
                    else:
                        nc.sync.dma_start(
                            out=t[32:, :],
                            in_=xqd_d[:, (b0 - 1) * XSTRIDE
                                      : (b0 + nb - 1) * XSTRIDE],
                        )
                    xts.append((t, b0))
                if stage_limit < 4:
                    continue

                def emit_copies(gi):
                    # reconstruct split gi's overlap partitions 0..31 on
                    # the DVE (partition-crossing copy, HW-verified; no
                    # DMA fabric traffic).  Interleaved into the compute
                    # loop so the DVE queue never stalls on loads.
                    t, _ = xts[gi]
                    nb = groups[gi][1]
                    if gi > 0:
                        pt, _ = xts[gi - 1]
                        pnb = groups[gi - 1][1]
                        nc.vector.tensor_copy(
                            out=t[:32, :XSTRIDE],
                            in_=pt[96:, (pnb - 1) * XSTRIDE : pnb * XSTRIDE],
                        )
                    if nb > 1:
                        nc.vector.tensor_copy(
                            out=t[:32, XSTRIDE : nb * XSTRIDE],
                            in_=t[96:, : (nb - 1) * XSTRIDE],
                        )

                emit_copies(0)
                emit_copies(1)

                st_tiles = []
                for w in range(N_WIN):
                    st_w = st_pool.tile([128, N_BLK * 96], I16, tag=f"st{w}",
                                        name=f"st{w}")
                    st_tiles.append(st_w)

                for gi, (b0, gn) in enumerate(groups):
                    if gi + 2 < len(groups):
                        emit_copies(gi + 2)
                    t, _ = xts[gi]
                    for w in range(N_WIN):
                        psy = ps_pool.tile([128, GRP, 96], F32, tag="psy")
                        for b in range(gn):
                            cb = XSTRIDE * b + 128 * w
                            for s in range(3):
                                nc.tensor.matmul(
                                    out=psy[:, b, :],
                                    lhsT=t[:, cb + s : cb + s + 128],
                                    rhs=wtiles[s][:],
                                    start=(s == 0),
                                    stop=(s == 2),
                                )
                        if stage_limit >= 6:
                            # floor -> int16, split DVE / ACT (both RNE
                            # casts, HW-verified) so neither engine
                            # out-paces the PE
                            if w in (1, 3, 5):
                                nc.scalar.activation(
                                    out=st_tiles[w][:, 96 * b0 : 96 * (b0 + gn)],
                                    in_=psy[:, :gn, :],
                                    func=mybir.ActivationFunctionType.Copy,
                                    bias=FLOOR_C, scale=1.0,
                                )
                            else:
                                nc.vector.tensor_scalar(
                                    out=st_tiles[w][:, 96 * b0 : 96 * (b0 + gn)],
                                    in0=psy[:, :gn, :],
                                    scalar1=FLOOR_C, scalar2=None, op0=add,
                                )
                    if stage_limit >= 7 and gi in (4, 6, 8):
                        # column waves 0:1920 / 1920:2880 / 2880:3552 are
                        # final for every window: stream them out.  Waves
                        # 1-2 overlap compute (gpsimd queue); the tail
                        # wave rides the faster HW-DGE scalar queue.
                        c0 = {4: 0, 6: 1920, 8: 2880}[gi]
                        c1 = 96 * (b0 + gn)
                        for w in range(N_WIN):
                            nc.gpsimd.dma_start(
                                out=y_d[128 * w : 128 * (w + 1), c0:c1],
                                in_=st_tiles[w][:, c0:c1],
                            )

    _split_multi_waits(nc)
    return nc


def _banded_weights(w: np.ndarray) -> np.ndarray:
    """w [3,3,16,16] f32 -> wb [3, 128, 96] fp16 banded lhsT matrices,
    PRE-SCALED by 1/256 (exact in fp16: just an exponent shift).

    wb[kh][16*dw + c, 16*j + k] = round(w*256)[kh, dw - j, c, k] / 256
    for 0 <= dw - j <= 2, j in 0..5."""
    wq = np.round(w.astype(np.float32) * np.float32(256.0))  # RNE, exact
    assert np.abs(wq).max() < 2048, "w_int exceeds fp16-exact budget"
    wb = np.zeros((3, 128, 96), dtype=np.float32)
    for kh in range(3):
        for j in range(6):
            for kw in range(3):
                dw = j + kw
                wb[kh, 16 * dw : 16 * dw + 16, 16 * j : 16 * j + 16] = wq[kh, kw]
    return (wb * np.float32(1.0 / 256.0)).astype(np.float16)


def _blocked_x(x: np.ndarray) -> tuple[np.ndarray, np.ndarray]:
    """x [32,224,224,16] f32 -> (xq0 [8, 128, XSTRIDE],
                                 xqd [8, 96, 36*XSTRIDE]) fp16, where
    stripe b partition 16*dw+c col 224*img+h holds
    round(x*256)[4*core+img, h, 6*b + dw, c]; xq0 is stripe 0 (full),
    xqd holds stripes 1..36 without their first 2 w-columns (those are
    reconstructed on-chip from the previous stripe).  Pad cols zero."""
    qx = np.round(x * np.float32(256.0)).astype(np.float16)  # RNE, exact
    sw = np.lib.stride_tricks.sliding_window_view(qx, 8, axis=2)
    sw = sw[:, :, ::6, :, :]                     # [32, 224h, 37b, 16c, 8dw]
    sw = sw.reshape(N_CORES, B_CORE, H, N_BLK, C, 8)
    xq = sw.transpose(0, 5, 4, 3, 1, 2)          # [8, 8dw, 16c, 37b, 4, 224]
    xq = np.ascontiguousarray(xq).reshape(N_CORES, 128, N_BLK, PACK)
    xq = np.pad(xq, ((0, 0), (0, 0), (0, 0), (0, XSTRIDE - PACK)))
    xq0 = np.ascontiguousarray(xq[:, :, 0, :])
    xqd = np.ascontiguousarray(xq[:, 32:, 1:, :]).reshape(
        N_CORES, 96, (N_BLK - 1) * XSTRIDE
    )
    return xq0, xqd


_RUNNER = None


def _get_runner():
    global _RUNNER
    if _RUNNER is None:
        _RUNNER = _make_runner(_build_nc())
    return _RUNNER


def _make_runner(nc):
    """Mirrors concourse.bass2jax.run_bass_via_pjrt's multi-core path but
    caches the jitted executable so repeated calls don't recompile."""
    import jax
    from jax.sharding import Mesh, PartitionSpec
    from jax.experimental.shard_map import shard_map
    from concourse.bass2jax import (
        _bass_exec_p,
        install_neuronx_cc_hook,
        partition_id_tensor,
    )

    install_neuronx_cc_hook()

    partition_name = nc.partition_id_tensor.name if nc.partition_id_tensor else None
    in_names, out_names, out_avals, zero_outs = [], [], [], []
    for alloc in nc.m.functions[0].allocations:
        if not isinstance(alloc, mybir.MemoryLocationSet):
            continue
        name = alloc.memorylocations[0].name
        if alloc.kind == "ExternalInput":
            if name != partition_name:
                in_names.append(name)
        elif alloc.kind == "ExternalOutput":
            out_names.append(name)
            shape = tuple(alloc.tensor_shape)
            dtype = mybir.dt.np(alloc.dtype)
            out_avals.append(jax.core.ShapedArray(shape, dtype))
            zero_outs.append(np.zeros(shape, dtype))
    n_params = len(in_names)
    n_outs = len(out_avals)
    all_in_names = list(in_names) + list(out_names)
    if partition_name is not None:
        all_in_names.append(partition_name)

    def _body(*args):
        operands = list(args)
        if partition_name is not None:
            operands.append(partition_id_tensor())
        outs = _bass_exec_p.bind(
            *operands,
            out_avals=tuple(out_avals),
            in_names=tuple(all_in_names),
            out_names=tuple(out_names),
            lowering_input_output_aliases=(),
            sim_require_finite=True,
            sim_require_nnan=True,
            nc=nc,
        )
        return tuple(outs)

    devices = jax.devices()[:N_CORES]
    assert len(devices) == N_CORES, f"need {N_CORES} devices, got {len(devices)}"
    mesh = Mesh(np.asarray(devices), ("core",))
    in_specs = (PartitionSpec("core"),) * (n_params + n_outs)
    out_specs = (PartitionSpec("core"),) * n_outs
    sharded = jax.jit(
        shard_map(_body, mesh=mesh, in_specs=in_specs, out_specs=out_specs,
                  check_rep=False),
        donate_argnums=tuple(range(n_params, n_params + n_outs)),
        keep_unused=True,
    )

    state = {
        "sharded": sharded,
        "in_names": in_names,
        "out_names": out_names,
        "out_avals": out_avals,
        "zero_outs": zero_outs,
        "n_cores": N_CORES,
    }

    def runner(in_maps):
        per_core = [[np.asarray(m[nm]) for nm in in_names] for m in in_maps]
        concat_in = [
            np.concatenate([per_core[c][i] for c in range(N_CORES)], axis=0)
            for i in range(n_params)
        ]
        concat_zeros = [
            np.concatenate([z] * N_CORES, axis=0) for z in zero_outs
        ]
        out_arrs = state["sharded"](*concat_in, *concat_zeros)
        return [
            {
                nm: np.asarray(out_arrs[i]).reshape(
                    N_CORES, *out_avals[i].shape
                )[c]
                for i, nm in enumerate(out_names)
            }
            for c in range(N_CORES)
        ]

    runner.state = state
    return runner


def _make_in_maps(x: np.ndarray, w: np.ndarray) -> list[dict]:
    wb = _banded_weights(np.asarray(w, dtype=np.float32))
    xq0, xqd = _blocked_x(np.asarray(x, dtype=np.float32))
    return [{"xq0": xq0[core], "xqd": xqd[core], "wb": wb}
            for core in range(N_CORES)]


def _assemble_out(results: list[dict]) -> np.ndarray:
    y16 = np.stack([r["y"] for r in results])  # [8, 896, 3552]
    y16 = y16.reshape(B_FULL, H, WO * K)[:, :HO]  # drop seam rows 222/223
    y = y16.astype(np.float32) * np.float32(1.0 / 256.0)
    return np.ascontiguousarray(y.reshape(B_FULL, HO, WO, K))


def kernel(x: np.ndarray, w: np.ndarray, fixed_point) -> np.ndarray:
    assert int(fixed_point) == 8, f"kernel hardcodes fixed_point=8, got {fixed_point}"
    x = np.ascontiguousarray(np.asarray(x, dtype=np.float32))
    assert x.shape == (B_FULL, H, W, C), x.shape
    assert np.abs(x).max() * 256.0 < 2040.0, "x_int exceeds fp16-exact budget"

    runner = _get_runner()
    results = runner(_make_in_maps(x, w))
    return _assemble_out(results)
